# revision 92
# baseline (speedup 1.0000x reference)
"""Trainium2 Bass kernel for nn_CFGEmbeder (masked attention pooling).

Reference (per sample, B=128, N=512 nodes, H=512):
    h      = tanh(code_feat @ W_sa + b_sa)          [N, H]
    scores = h @ w_sc (+ b_sc)                      [N]
    attn   = softmax(scores over valid nodes)       [N]
    out    = tanh(attn @ code_feat)                 [H]

Only ~50% of nodes are valid (node_mask); the reference computes the rest
and discards them.  This kernel packs the valid nodes host-side so the
device only touches real work:

  * Samples are sorted by valid count and dealt round-robin to the 8 cores
    (rank r -> core r%8, slot r//8), so the same slot widths work on every
    core and can be baked into the single SPMD program.  Slots are grouped
    in quarters of 4; within a quarter all slots are padded to the same
    width (the quarter max, ~1% extra), keeping every access pattern
    regular.  The host un-shuffles output rows at the end.
  * b_sc is dropped (softmax shift invariance).  b_sa==0 takes a fused
    wide-ACT path; nonzero b_sa falls back to per-m-chunk ACTs with bias.
  * No max-subtraction in softmax: |scores| <= ||w_sc||_1 * max|tanh| so
    exp stays comfortably inside f32 range, and masked positions use the
    shift-invariant (s+1000)*mask trick whose exp underflows cleanly to 0.

Device pipeline, one slot (node-packed sample) at a time, fp16 matmuls
with f32 PSUM:

  mm1    hT[m, i] = sum_k W[k,m].T xT[k,i] over the slot's columns; tanh
         fused on ScalarE over 2-bank psum pairs -> th fp16.
  score  M=1 matvecs (1-col LDWEIGHTS is ~free): slot j's row accumulates
         at psum partition 32*(j%4) of its quarter's score tile via
         col-tiling (tile_position=(0,32*(j%4))) -- scores land spread
         across partitions with NO cross-partition move and NO DRAM
         bounce.  Engines only pay free-dim cost, so the softmax runs on
         the partition-strided [97, smax] view directly; garbage rows are
         masked to 0 (then exp(0-1000) == 0).
  smax   (s+1000)*mask on DVE, exp with accumulate on ScalarE, recip+mul.
  pool   attn -> PE transpose -> attnT columns; out[s] = sum_c
         attnT[:,c,32j].T @ xnat[s,c] with 4 samples per psum bank via
         col-tiling; fused tanh on the whole bank, DMA of the 4 rows.

Quarters are software-pipelined: each quarter's softmax+pooling tail is
emitted into the next quarter's matmul stream, so only the last quarter's
short (DMA-free) tail is exposed at the end.
"""

from contextlib import ExitStack

import numpy as np

import concourse.bass as bass
import concourse.bacc as bacc
import concourse.mybir as mybir
import concourse.tile as tile
from concourse.bass_utils import run_bass_kernel_spmd

F16 = mybir.dt.float16
F32 = mybir.dt.float32

B, N, H = 128, 512, 512
NCORES = 8
S = B // NCORES          # 16 samples (slots) per core
NQ = S // 4              # 4 quarters of 4 slots
KC = H // 128            # contraction chunks
MC = H // 128            # m chunks
SHIFT = 1000.0

Tanh = mybir.ActivationFunctionType.Tanh
Exp = mybir.ActivationFunctionType.Exp
Alu = mybir.AluOpType


def make_plan(node_mask):
    """Slot assignment + per-quarter widths (shared across cores)."""
    k = node_mask.astype(bool).sum(1)
    order = np.argsort(-k, kind="stable")
    qw = []
    for q in range(NQ):
        grp = k[order[q * 4 * NCORES:(q + 1) * 4 * NCORES]]
        qw.append(max(16, int(np.ceil(grp.max() / 16) * 16)))
    ncc = [(w + 127) // 128 for w in qw]
    return dict(order=order, qw=qw, ncc=ncc,
                cj_off=np.concatenate([[0], np.cumsum(np.repeat(ncc, 4))])
                .astype(int))


def build_program(plan, bsa_zero):
    qw = plan["qw"]
    ncc = plan["ncc"]
    cj_off = plan["cj_off"]
    cj_tot = int(cj_off[-1])

    nc = bacc.Bacc(trn_type="TRN2", target_bir_lowering=False,
                   num_devices=NCORES)

    # f32 const blob columns: bsa | ident97 | per-quarter masks (97 rows)
    CB_BSA, CB_ID = 0, MC
    CB_MSK = [CB_ID + 97]
    for q in range(NQ):
        CB_MSK.append(CB_MSK[-1] + qw[q])
    CB = CB_MSK[-1]

    xt_h = [nc.dram_tensor(f"xt{q}", [128, 4, KC, qw[q]], F16,
                           kind="ExternalInput") for q in range(NQ)]
    xn_h = [nc.dram_tensor(f"xn{q}", [128, 4 * ncc[q], H], F16,
                           kind="ExternalInput") for q in range(NQ)]
    wb_h = nc.dram_tensor("wblob", [128, MC * H + MC], F16,
                          kind="ExternalInput")
    cb_h = nc.dram_tensor("cblob", [128, CB], F32, kind="ExternalInput")
    out_h = nc.dram_tensor("out", [S, H], F32, kind="ExternalOutput")

    with tile.TileContext(nc) as tc, ExitStack() as ctx:
        const = ctx.enter_context(tc.tile_pool(name="const", bufs=1))
        xt_p = ctx.enter_context(tc.tile_pool(name="xt", bufs=1))
        xn_p = ctx.enter_context(tc.tile_pool(name="xn", bufs=1))
        th_p = ctx.enter_context(tc.tile_pool(name="th", bufs=6))
        sm_p = ctx.enter_context(tc.tile_pool(name="sm", bufs=1))
        row_p = ctx.enter_context(tc.tile_pool(name="row", bufs=2))
        ph_p = ctx.enter_context(tc.tile_pool(name="ph", bufs=5, space="PSUM"))
        pr_p = ctx.enter_context(tc.tile_pool(name="pr", bufs=2, space="PSUM"))
        pt_p = ctx.enter_context(tc.tile_pool(name="pt", bufs=1, space="PSUM"))

        # --- constants: few large DMAs (a dma_start costs ~0.6us of
        # issuing-engine time); weight blob split so m0 lands first
        wb = const.tile([128, MC * H + MC], F16, name="wb")
        cb = const.tile([128, CB], F32, name="cb")
        nc.scalar.dma_start(wb, wb_h.ap())
        nc.scalar.dma_start(cb, cb_h.ap())

        def Wf(k, m):
            return wb[:, m * H + k * 128:m * H + (k + 1) * 128]

        wsc = wb[:, MC * H:]
        bsa = cb[:, CB_BSA:CB_BSA + MC]
        idf = cb[:, CB_ID:CB_ID + 97]
        masks = [cb[0:97, CB_MSK[q]:CB_MSK[q + 1]] for q in range(NQ)]

        # HAM warmup: garbage matmuls on a memset tile (no DMA dependency)
        # keep PE activity continuous from ~7us into the real mm1 stream,
        # so the 2.4 GHz un-throttle fires before it starts
        wz = const.tile([128, 128], F16, name="wz")
        nc.vector.memset(wz, 0.0)
        nshift = const.tile([97, 1], F32, name="nshift")
        nc.vector.memset(nshift, -SHIFT)
        warm = pt_p.tile([128, H], F32, name="pp")
        for i in range(40):
            nc.tensor.matmul(warm[:, 0:128], lhsT=wz, rhs=wz,
                             start=(i == 0), stop=(i == 39))

        # --- x loads: per-quarter tensors, spread so each arrives well
        # before its quarter is consumed and xn never starves xt
        xt = [xt_p.tile([128, 4, KC, qw[q]], F16, name=f"xt{q}")
              for q in range(NQ)]
        xn = [xn_p.tile([128, 4 * ncc[q], H], F16, name=f"xn{q}")
              for q in range(NQ)]
        # each ~110GB/s ring carries transfers in consumption order; the
        # zero tail of each slot's last 128-node chunk is never sent (the
        # pooling matmul contracts only the first rem partitions there)
        def xn_load(q, queue):
            nf = ncc[q] - 1
            rem = qw[q] - 128 * nf
            if nf:
                queue.dma_start(
                    xn[q].rearrange("p (s c) h -> p s c h", c=ncc[q])
                    [:, :, 0:nf, :],
                    xn_h[q].ap().rearrange("p (s c) h -> p s c h", c=ncc[q])
                    [:, :, 0:nf, :])
            queue.dma_start(
                xn[q].rearrange("p (s c) h -> p s c h", c=ncc[q])
                [0:rem, :, nf, :],
                xn_h[q].ap().rearrange("p (s c) h -> p s c h", c=ncc[q])
                [0:rem, :, nf, :])

        # quarter 0 as per-slot singles on gpsimd (first bytes land ~10us);
        # later quarters split 2+2 across the fast rings in deadline order;
        # the slow sync ring carries only the two latest-needed xn blocks
        for jj in range(4):
            nc.gpsimd.dma_start(xt[0][:, jj], xt_h[0].ap()[:, jj])
        nc.sync.dma_start(xn[0], xn_h[0].ap())
        for q in (1, 2, 3):
            nc.scalar.dma_start(xt[q][:, 0:2], xt_h[q].ap()[:, 0:2])
            nc.gpsimd.dma_start(xt[q][:, 2:4], xt_h[q].ap()[:, 2:4])
        xn_load(1, nc.scalar)
        xn_load(3, nc.gpsimd)
        xn_load(2, nc.sync)

        def slot_mm(q, jj):
            """mm1 + fused tanh for one slot; returns the th tiles."""
            w = qw[q]
            ths = []
            for pair in range(2):
                th = th_p.tile([128, 2, 288], F16, name="th")
                for mi in range(2):
                    m = 2 * pair + mi
                    # one bank per m-chunk, 5 rotating buffers: the PE can
                    # run >1 m-group ahead of the tanh ACTs, hiding the
                    # psum-recycle semaphore waits seen as ~200ns gaps
                    ph = ph_p.tile([128, 512], F32, name="ph")
                    for k in range(KC):
                        nc.tensor.matmul(
                            ph[:, 0:w],
                            lhsT=Wf(k, m),
                            rhs=xt[q][:, jj, k, :],
                            start=(k == 0), stop=(k == KC - 1),
                        )
                    bias = 0.0 if bsa_zero else bsa[:, m:m + 1]
                    nc.scalar.activation(th[:, mi, 0:w], ph[:, 0:w],
                                         Tanh, bias=bias)
                ths.append(th)
            return ths

        def slot_scores(q, jj, ths, pscore):
            """col-tiled score row; deferred one slot behind its mm1 so
            the PE never stalls on the tanh ACTs it reads."""
            w = qw[q]
            for m in range(MC):
                nc.tensor.matmul(
                    pscore[32 * jj:32 * jj + 1, 0:w],
                    lhsT=wsc[:, m:m + 1],
                    rhs=ths[m // 2][:, m % 2, 0:w],
                    start=(m == 0), stop=(m == MC - 1),
                    tile_position=(0, 32 * jj),
                )

        def tail_soft(q, pscore):
            """softmax on the partition-strided score tile (no PE work).
            Rows between the 4 live ones hold garbage; the mask zeroes
            them ((g+SHIFT)*0 = 0) and exp(0-SHIFT) underflows to 0."""
            w = qw[q]
            sco = sm_p.tile([97, w], F32, name=f"sco{q}")
            nc.vector.tensor_copy(sco, pscore[0:97, 0:w])
            m1 = sm_p.tile([97, w], F32, name=f"m1_{q}")
            nc.vector.scalar_tensor_tensor(m1, sco, SHIFT, masks[q],
                                           op0=Alu.add, op1=Alu.mult)
            ex = sm_p.tile([97, w], F32, name=f"ex{q}")
            esum = sm_p.tile([97, 1], F32, name=f"esum{q}")
            nc.scalar.activation(ex, m1, Exp, bias=nshift, accum_out=esum)
            rinv = sm_p.tile([97, 1], F32, name=f"rinv{q}")
            nc.vector.reciprocal(rinv, esum)
            attn = sm_p.tile([97, w], F32, name=f"attn{q}")
            nc.vector.tensor_scalar_mul(attn, ex, rinv)
            return attn

        def tail_pool(q, attn):
            """attn transpose + col-tiled pooling + output (PE phase)."""
            w = qw[q]
            paT = pt_p.tile([128, H], F32, name="pp")
            for c in range(ncc[q]):
                wcol = min(128, w - c * 128)
                nc.tensor.transpose(paT[0:wcol, c * 97:c * 97 + 97],
                                    attn[:, c * 128:c * 128 + wcol],
                                    idf[0:97, :])
            attnT = sm_p.tile([128, ncc[q], 97], F16, name=f"attnT{q}")
            nc.vector.tensor_copy(attnT, paT[:, 0:ncc[q] * 97])

            pp = pt_p.tile([128, H], F32, name="pp")
            for c in range(ncc[q]):
                # last chunk contracts only its real rem partitions (the
                # zero tail is neither loaded nor touched)
                pn = min(128, w - 128 * c)
                for jj in range(4):
                    nc.tensor.matmul(
                        pp[32 * jj:32 * jj + 1, :],
                        lhsT=attnT[0:pn, c, 32 * jj:32 * jj + 1],
                        rhs=xn[q][0:pn, jj * ncc[q] + c, :],
                        start=(c == 0), stop=(c == ncc[q] - 1),
                        tile_position=(0, 32 * jj),
                    )
            orow = row_p.tile([128, H], F32, name="orow")
            nc.scalar.activation(orow, pp, Tanh)
            nc.sync.dma_start(
                out_h.ap().rearrange("(q four) h -> four q h", four=4)
                [:, q, :],
                orow[0:97:32, :])

        # pipeline: slot scores trail their mm1 by one slot, a quarter's
        # softmax follows its last scores, and its PE pooling is deferred
        # into the middle of the next quarter's matmul stream
        # pooling runs two quarters behind its softmax so the xn transfers
        # (on the slower rings) have relaxed deadlines
        attns = {}
        pscores = {}
        prev = None
        ready = []
        for q in range(NQ):
            pscores[q] = pr_p.tile([128, 512], F32, name="pscore")
            for jj in range(4):
                ths = slot_mm(q, jj)
                if prev is not None:
                    pq, pjj, pth = prev
                    slot_scores(pq, pjj, pth, pscores[pq])
                    if pjj == 3:
                        attns[pq] = tail_soft(pq, pscores.pop(pq))
                        ready.append(pq)
                prev = (q, jj, ths)
                if jj == 2 and ready:
                    pq2 = ready.pop(0)
                    tail_pool(pq2, attns.pop(pq2))
        q, jj, ths = prev
        slot_scores(q, jj, ths, pscores[q])
        attns[q] = tail_soft(q, pscores.pop(q))
        ready.append(q)
        for pq in ready:
            tail_pool(pq, attns.pop(pq))

    nc.finalize()
    return nc


_CACHE = {}


def _get_nc(plan, bsa_zero):
    key = (tuple(plan["qw"]), bsa_zero)
    if key not in _CACHE:
        _CACHE[key] = build_program(plan, bsa_zero)
    return _CACHE[key]


def make_in_maps(plan, code_feat, node_mask, W_sa, b_sa, w_sc):
    x16 = np.asarray(code_feat, dtype=np.float16)
    kmask = np.asarray(node_mask).astype(bool)
    order = plan["order"]
    qw, ncc = plan["qw"], plan["ncc"]

    w16 = np.asarray(W_sa, dtype=np.float16)
    wblob = np.empty((128, MC * H + MC), dtype=np.float16)
    wblob[:, 0:MC * H] = (w16.reshape(KC, 128, MC, 128)
                          .transpose(1, 2, 0, 3).reshape(128, MC * H))
    wblob[:, MC * H:] = np.asarray(w_sc, dtype=np.float16).reshape(MC, 128).T

    CB = MC + 97 + sum(qw)
    cblob0 = np.zeros((128, CB), dtype=np.float32)
    cblob0[:, 0:MC] = np.asarray(b_sa, dtype=np.float32).reshape(MC, 128).T
    cblob0[0:97, MC:MC + 97] = np.eye(97, dtype=np.float32)
    qcol = np.concatenate([[MC + 97], MC + 97 + np.cumsum(qw)]).astype(int)

    in_maps = []
    for i in range(NCORES):
        im = {"wblob": wblob}
        cblob = cblob0.copy()
        for q in range(NQ):
            w, nc_q = qw[q], ncc[q]
            xtq = np.zeros((4, w, H), dtype=np.float16)
            xnq = np.zeros((4, nc_q * 128, H), dtype=np.float16)
            for jj in range(4):
                j = q * 4 + jj
                s = order[j * NCORES + i]
                idx = np.nonzero(kmask[s])[0]
                xtq[jj, 0:len(idx)] = x16[s, idx]
                xnq[jj, 0:len(idx)] = x16[s, idx]
                cblob[32 * jj, qcol[q]:qcol[q] + len(idx)] = 1.0
            # xt: [p(h%128), slot, k, i]
            im[f"xt{q}"] = np.ascontiguousarray(
                xtq.transpose(0, 2, 1).reshape(4, KC, 128, w)
                .transpose(2, 0, 1, 3))
            # xn: [p(n%128), slot*ncc + c, h]
            im[f"xn{q}"] = np.ascontiguousarray(
                xnq.reshape(4 * nc_q, 128, H).transpose(1, 0, 2))
            im["cblob"] = cblob
        in_maps.append(im)
    return in_maps


def kernel(code_feat, node_mask, W_sa, b_sa, w_sc, b_sc=None, **_ignored):
    code_feat = np.asarray(code_feat)
    node_mask = np.asarray(node_mask)
    plan = make_plan(node_mask)
    bsa_zero = not np.any(np.asarray(b_sa))
    nc = _get_nc(plan, bsa_zero)
    in_maps = make_in_maps(plan, code_feat, node_mask, W_sa, b_sa, w_sc)
    res = run_bass_kernel_spmd(nc, in_maps, list(range(NCORES)))
    out = np.empty((B, H), dtype=np.float32)
    order = plan["order"]
    for i in range(NCORES):
        for j in range(S):
            out[order[j * NCORES + i]] = res.results[i]["out"][j]
    return out


# revision 94
# speedup vs baseline: 1.0072x; 1.0072x over previous
"""Trainium2 Bass kernel for nn_CFGEmbeder (masked attention pooling).

Reference (per sample, B=128, N=512 nodes, H=512):
    h      = tanh(code_feat @ W_sa + b_sa)          [N, H]
    scores = h @ w_sc (+ b_sc)                      [N]
    attn   = softmax(scores over valid nodes)       [N]
    out    = tanh(attn @ code_feat)                 [H]

Only ~50% of nodes are valid (node_mask); the reference computes the rest
and discards them.  This kernel packs the valid nodes host-side so the
device only touches real work:

  * Samples are sorted by valid count and dealt round-robin to the 8 cores
    (rank r -> core r%8, slot r//8), so the same slot widths work on every
    core and can be baked into the single SPMD program.  Slots are grouped
    in quarters of 4; within a quarter all slots are padded to the same
    width (the quarter max, ~1% extra), keeping every access pattern
    regular.  The host un-shuffles output rows at the end.
  * b_sc is dropped (softmax shift invariance).  b_sa==0 takes a fused
    wide-ACT path; nonzero b_sa falls back to per-m-chunk ACTs with bias.
  * No max-subtraction in softmax: |scores| <= ||w_sc||_1 * max|tanh| so
    exp stays comfortably inside f32 range, and masked positions use the
    shift-invariant (s+1000)*mask trick whose exp underflows cleanly to 0.

Device pipeline, one slot (node-packed sample) at a time, fp16 matmuls
with f32 PSUM:

  mm1    hT[m, i] = sum_k W[k,m].T xT[k,i] over the slot's columns; tanh
         fused on ScalarE over 2-bank psum pairs -> th fp16.
  score  M=1 matvecs (1-col LDWEIGHTS is ~free): slot j's row accumulates
         at psum partition 32*(j%4) of its quarter's score tile via
         col-tiling (tile_position=(0,32*(j%4))) -- scores land spread
         across partitions with NO cross-partition move and NO DRAM
         bounce.  Engines only pay free-dim cost, so the softmax runs on
         the partition-strided [97, smax] view directly; garbage rows are
         masked to 0 (then exp(0-1000) == 0).
  smax   (s+1000)*mask on DVE, exp with accumulate on ScalarE, recip+mul.
  pool   attn -> PE transpose -> attnT columns; out[s] = sum_c
         attnT[:,c,32j].T @ xnat[s,c] with 4 samples per psum bank via
         col-tiling; fused tanh on the whole bank, DMA of the 4 rows.

Quarters are software-pipelined: each quarter's softmax+pooling tail is
emitted into the next quarter's matmul stream, so only the last quarter's
short (DMA-free) tail is exposed at the end.
"""

from contextlib import ExitStack

import numpy as np

import concourse.bass as bass
import concourse.bacc as bacc
import concourse.mybir as mybir
import concourse.tile as tile
from concourse.bass_utils import run_bass_kernel_spmd

F16 = mybir.dt.float16
F32 = mybir.dt.float32

B, N, H = 128, 512, 512
NCORES = 8
S = B // NCORES          # 16 samples (slots) per core
NQ = S // 4              # 4 quarters of 4 slots
KC = H // 128            # contraction chunks
MC = H // 128            # m chunks
SHIFT = 1000.0

Tanh = mybir.ActivationFunctionType.Tanh
Exp = mybir.ActivationFunctionType.Exp
Alu = mybir.AluOpType


def make_plan(node_mask):
    """Slot assignment + per-quarter widths (shared across cores)."""
    k = node_mask.astype(bool).sum(1)
    order = np.argsort(-k, kind="stable")
    qw = []
    for q in range(NQ):
        grp = k[order[q * 4 * NCORES:(q + 1) * 4 * NCORES]]
        qw.append(max(16, int(np.ceil(grp.max() / 16) * 16)))
    ncc = [(w + 127) // 128 for w in qw]
    return dict(order=order, qw=qw, ncc=ncc,
                cj_off=np.concatenate([[0], np.cumsum(np.repeat(ncc, 4))])
                .astype(int))


def build_program(plan, bsa_zero):
    qw = plan["qw"]
    ncc = plan["ncc"]
    cj_off = plan["cj_off"]
    cj_tot = int(cj_off[-1])

    nc = bacc.Bacc(trn_type="TRN2", target_bir_lowering=False,
                   num_devices=NCORES)

    # f32 const blob columns: bsa | ident97 | per-quarter masks (97 rows)
    CB_BSA, CB_ID = 0, MC
    CB_MSK = [CB_ID + 97]
    for q in range(NQ):
        CB_MSK.append(CB_MSK[-1] + qw[q])
    CB = CB_MSK[-1]

    xt_h = [nc.dram_tensor(f"xt{q}", [128, 4, KC, qw[q]], F16,
                           kind="ExternalInput") for q in range(NQ)]
    xn_h = [nc.dram_tensor(f"xn{q}", [128, 4 * ncc[q], H], F16,
                           kind="ExternalInput") for q in range(NQ)]
    wb_h = nc.dram_tensor("wblob", [128, MC * H + MC], F16,
                          kind="ExternalInput")
    cb_h = nc.dram_tensor("cblob", [128, CB], F32, kind="ExternalInput")
    out_h = nc.dram_tensor("out", [S, H], F32, kind="ExternalOutput")

    with tile.TileContext(nc) as tc, ExitStack() as ctx:
        const = ctx.enter_context(tc.tile_pool(name="const", bufs=1))
        xt_p = ctx.enter_context(tc.tile_pool(name="xt", bufs=1))
        xn_p = ctx.enter_context(tc.tile_pool(name="xn", bufs=1))
        th_p = ctx.enter_context(tc.tile_pool(name="th", bufs=6))
        sm_p = ctx.enter_context(tc.tile_pool(name="sm", bufs=1))
        row_p = ctx.enter_context(tc.tile_pool(name="row", bufs=2))
        ph_p = ctx.enter_context(tc.tile_pool(name="ph", bufs=2, space="PSUM"))
        pr_p = ctx.enter_context(tc.tile_pool(name="pr", bufs=2, space="PSUM"))
        pt_p = ctx.enter_context(tc.tile_pool(name="pt", bufs=2, space="PSUM"))

        # --- constants: few large DMAs (a dma_start costs ~0.6us of
        # issuing-engine time); weight blob split so m0 lands first
        wb = const.tile([128, MC * H + MC], F16, name="wb")
        cb = const.tile([128, CB], F32, name="cb")
        nc.scalar.dma_start(wb, wb_h.ap())
        nc.scalar.dma_start(cb, cb_h.ap())

        def Wf(k, m):
            return wb[:, m * H + k * 128:m * H + (k + 1) * 128]

        wsc = wb[:, MC * H:]
        bsa = cb[:, CB_BSA:CB_BSA + MC]
        idf = cb[:, CB_ID:CB_ID + 97]
        masks = [cb[0:97, CB_MSK[q]:CB_MSK[q + 1]] for q in range(NQ)]

        # HAM warmup: garbage matmuls on a memset tile (no DMA dependency)
        # keep PE activity continuous from ~7us into the real mm1 stream,
        # so the 2.4 GHz un-throttle fires before it starts
        wz = const.tile([128, 128], F16, name="wz")
        nc.vector.memset(wz, 0.0)
        nshift = const.tile([97, 1], F32, name="nshift")
        nc.vector.memset(nshift, -SHIFT)
        warm = pt_p.tile([128, H], F32, name="pp")
        for i in range(40):
            nc.tensor.matmul(warm[:, 0:128], lhsT=wz, rhs=wz,
                             start=(i == 0), stop=(i == 39))

        # --- x loads: per-quarter tensors, spread so each arrives well
        # before its quarter is consumed and xn never starves xt
        xt = [xt_p.tile([128, 4, KC, qw[q]], F16, name=f"xt{q}")
              for q in range(NQ)]
        xn = [xn_p.tile([128, 4 * ncc[q], H], F16, name=f"xn{q}")
              for q in range(NQ)]
        # each ~110GB/s ring carries transfers in consumption order; the
        # zero tail of each slot's last 128-node chunk is never sent (the
        # pooling matmul contracts only the first rem partitions there)
        def xn_load(q, queue):
            nf = ncc[q] - 1
            rem = qw[q] - 128 * nf
            if nf:
                queue.dma_start(
                    xn[q].rearrange("p (s c) h -> p s c h", c=ncc[q])
                    [:, :, 0:nf, :],
                    xn_h[q].ap().rearrange("p (s c) h -> p s c h", c=ncc[q])
                    [:, :, 0:nf, :])
            queue.dma_start(
                xn[q].rearrange("p (s c) h -> p s c h", c=ncc[q])
                [0:rem, :, nf, :],
                xn_h[q].ap().rearrange("p (s c) h -> p s c h", c=ncc[q])
                [0:rem, :, nf, :])

        # quarter 0 as per-slot singles on gpsimd (first bytes land ~10us);
        # later quarters split 2+2 across the fast rings in deadline order;
        # the slow sync ring carries only the two latest-needed xn blocks
        for jj in range(4):
            nc.gpsimd.dma_start(xt[0][:, jj], xt_h[0].ap()[:, jj])
        nc.sync.dma_start(xn[0], xn_h[0].ap())
        for q in (1, 2, 3):
            nc.scalar.dma_start(xt[q][:, 0:2], xt_h[q].ap()[:, 0:2])
            nc.gpsimd.dma_start(xt[q][:, 2:4], xt_h[q].ap()[:, 2:4])
        xn_load(1, nc.scalar)
        xn_load(3, nc.gpsimd)
        xn_load(2, nc.sync)

        def slot_mm(q, jj):
            """mm1 + fused tanh for one slot; returns the th tiles."""
            w = qw[q]
            ths = []
            for pair in range(2):
                # mi stride padded to 512 so each matmul's output stays
                # inside one 2KB PSUM bank -- a matmul that straddles a
                # bank boundary silently corrupts the spill region
                ph = ph_p.tile([128, 2, 512], F32, name="ph")
                for mi in range(2):
                    m = 2 * pair + mi
                    for k in range(KC):
                        nc.tensor.matmul(
                            ph[:, mi, 0:w],
                            lhsT=Wf(k, m),
                            rhs=xt[q][:, jj, k, :],
                            start=(k == 0), stop=(k == KC - 1),
                        )
                th = th_p.tile([128, 2, 288], F16, name="th")
                if bsa_zero:
                    nc.scalar.activation(th[:, :, 0:w], ph[:, :, 0:w], Tanh)
                else:
                    for mi in range(2):
                        m = 2 * pair + mi
                        nc.scalar.activation(th[:, mi, 0:w], ph[:, mi, 0:w],
                                             Tanh, bias=bsa[:, m:m + 1])
                ths.append(th)
            return ths

        def slot_scores(q, jj, ths, pscore):
            """col-tiled score row; deferred one slot behind its mm1 so
            the PE never stalls on the tanh ACTs it reads."""
            w = qw[q]
            for m in range(MC):
                nc.tensor.matmul(
                    pscore[32 * jj:32 * jj + 1, 0:w],
                    lhsT=wsc[:, m:m + 1],
                    rhs=ths[m // 2][:, m % 2, 0:w],
                    start=(m == 0), stop=(m == MC - 1),
                    tile_position=(0, 32 * jj),
                )

        def tail_soft(q, pscore):
            """softmax on the partition-strided score tile (no PE work).
            Rows between the 4 live ones hold garbage; the mask zeroes
            them ((g+SHIFT)*0 = 0) and exp(0-SHIFT) underflows to 0."""
            w = qw[q]
            sco = sm_p.tile([97, w], F32, name=f"sco{q}")
            nc.vector.tensor_copy(sco, pscore[0:97, 0:w])
            m1 = sm_p.tile([97, w], F32, name=f"m1_{q}")
            nc.vector.scalar_tensor_tensor(m1, sco, SHIFT, masks[q],
                                           op0=Alu.add, op1=Alu.mult)
            ex = sm_p.tile([97, w], F32, name=f"ex{q}")
            esum = sm_p.tile([97, 1], F32, name=f"esum{q}")
            nc.scalar.activation(ex, m1, Exp, bias=nshift, accum_out=esum)
            rinv = sm_p.tile([97, 1], F32, name=f"rinv{q}")
            nc.vector.reciprocal(rinv, esum)
            attn = sm_p.tile([97, w], F32, name=f"attn{q}")
            nc.vector.tensor_scalar_mul(attn, ex, rinv)
            return attn

        def tail_pool(q, attn):
            """attn transpose + col-tiled pooling + output (PE phase)."""
            w = qw[q]
            paT = pt_p.tile([128, H], F32, name="pp")
            for c in range(ncc[q]):
                wcol = min(128, w - c * 128)
                nc.tensor.transpose(paT[0:wcol, c * 97:c * 97 + 97],
                                    attn[:, c * 128:c * 128 + wcol],
                                    idf[0:97, :])
            attnT = sm_p.tile([128, ncc[q], 97], F16, name=f"attnT{q}")
            nc.vector.tensor_copy(attnT, paT[:, 0:ncc[q] * 97])

            pp = pt_p.tile([128, H], F32, name="pp")
            for c in range(ncc[q]):
                # last chunk contracts only its real rem partitions (the
                # zero tail is neither loaded nor touched)
                pn = min(128, w - 128 * c)
                for jj in range(4):
                    nc.tensor.matmul(
                        pp[32 * jj:32 * jj + 1, :],
                        lhsT=attnT[0:pn, c, 32 * jj:32 * jj + 1],
                        rhs=xn[q][0:pn, jj * ncc[q] + c, :],
                        start=(c == 0), stop=(c == ncc[q] - 1),
                        tile_position=(0, 32 * jj),
                    )
            orow = row_p.tile([128, H], F32, name="orow")
            nc.scalar.activation(orow, pp, Tanh)
            nc.sync.dma_start(
                out_h.ap().rearrange("(q four) h -> four q h", four=4)
                [:, q, :],
                orow[0:97:32, :])

        # pipeline: slot scores trail their mm1 by one slot, a quarter's
        # softmax follows its last scores, and its PE pooling is deferred
        # into the middle of the next quarter's matmul stream
        # pooling runs two quarters behind its softmax so the xn transfers
        # (on the slower rings) have relaxed deadlines
        attns = {}
        pscores = {}
        prev = None
        ready = []
        for q in range(NQ):
            pscores[q] = pr_p.tile([128, 512], F32, name="pscore")
            for jj in range(4):
                ths = slot_mm(q, jj)
                if prev is not None:
                    pq, pjj, pth = prev
                    slot_scores(pq, pjj, pth, pscores[pq])
                    if pjj == 3:
                        attns[pq] = tail_soft(pq, pscores.pop(pq))
                        ready.append(pq)
                prev = (q, jj, ths)
                if jj == 2 and ready:
                    pq2 = ready.pop(0)
                    tail_pool(pq2, attns.pop(pq2))
        q, jj, ths = prev
        slot_scores(q, jj, ths, pscores[q])
        attns[q] = tail_soft(q, pscores.pop(q))
        ready.append(q)
        for pq in ready:
            tail_pool(pq, attns.pop(pq))

    nc.finalize()
    return nc


_CACHE = {}


def _get_nc(plan, bsa_zero):
    key = (tuple(plan["qw"]), bsa_zero)
    if key not in _CACHE:
        _CACHE[key] = build_program(plan, bsa_zero)
    return _CACHE[key]


def make_in_maps(plan, code_feat, node_mask, W_sa, b_sa, w_sc):
    x16 = np.asarray(code_feat, dtype=np.float16)
    kmask = np.asarray(node_mask).astype(bool)
    order = plan["order"]
    qw, ncc = plan["qw"], plan["ncc"]

    w16 = np.asarray(W_sa, dtype=np.float16)
    wblob = np.empty((128, MC * H + MC), dtype=np.float16)
    wblob[:, 0:MC * H] = (w16.reshape(KC, 128, MC, 128)
                          .transpose(1, 2, 0, 3).reshape(128, MC * H))
    wblob[:, MC * H:] = np.asarray(w_sc, dtype=np.float16).reshape(MC, 128).T

    CB = MC + 97 + sum(qw)
    cblob0 = np.zeros((128, CB), dtype=np.float32)
    cblob0[:, 0:MC] = np.asarray(b_sa, dtype=np.float32).reshape(MC, 128).T
    cblob0[0:97, MC:MC + 97] = np.eye(97, dtype=np.float32)
    qcol = np.concatenate([[MC + 97], MC + 97 + np.cumsum(qw)]).astype(int)

    in_maps = []
    for i in range(NCORES):
        im = {"wblob": wblob}
        cblob = cblob0.copy()
        for q in range(NQ):
            w, nc_q = qw[q], ncc[q]
            xtq = np.zeros((4, w, H), dtype=np.float16)
            xnq = np.zeros((4, nc_q * 128, H), dtype=np.float16)
            for jj in range(4):
                j = q * 4 + jj
                s = order[j * NCORES + i]
                idx = np.nonzero(kmask[s])[0]
                xtq[jj, 0:len(idx)] = x16[s, idx]
                xnq[jj, 0:len(idx)] = x16[s, idx]
                cblob[32 * jj, qcol[q]:qcol[q] + len(idx)] = 1.0
            # xt: [p(h%128), slot, k, i]
            im[f"xt{q}"] = np.ascontiguousarray(
                xtq.transpose(0, 2, 1).reshape(4, KC, 128, w)
                .transpose(2, 0, 1, 3))
            # xn: [p(n%128), slot*ncc + c, h]
            im[f"xn{q}"] = np.ascontiguousarray(
                xnq.reshape(4 * nc_q, 128, H).transpose(1, 0, 2))
            im["cblob"] = cblob
        in_maps.append(im)
    return in_maps


def kernel(code_feat, node_mask, W_sa, b_sa, w_sc, b_sc=None, **_ignored):
    code_feat = np.asarray(code_feat)
    node_mask = np.asarray(node_mask)
    plan = make_plan(node_mask)
    bsa_zero = not np.any(np.asarray(b_sa))
    nc = _get_nc(plan, bsa_zero)
    in_maps = make_in_maps(plan, code_feat, node_mask, W_sa, b_sa, w_sc)
    res = run_bass_kernel_spmd(nc, in_maps, list(range(NCORES)))
    out = np.empty((B, H), dtype=np.float32)
    order = plan["order"]
    for i in range(NCORES):
        for j in range(S):
            out[order[j * NCORES + i]] = res.results[i]["out"][j]
    return out


# revision 98
# speedup vs baseline: 1.0485x; 1.0410x over previous
"""Trainium2 Bass kernel for nn_CFGEmbeder (masked attention pooling).

Reference (per sample, B=128, N=512 nodes, H=512):
    h      = tanh(code_feat @ W_sa + b_sa)          [N, H]
    scores = h @ w_sc (+ b_sc)                      [N]
    attn   = softmax(scores over valid nodes)       [N]
    out    = tanh(attn @ code_feat)                 [H]

Only ~50% of nodes are valid (node_mask); the reference computes the rest
and discards them.  This kernel packs the valid nodes host-side so the
device only touches real work:

  * Samples are sorted by valid count and dealt round-robin to the 8 cores
    (rank r -> core r%8, slot r//8), so the same slot widths work on every
    core and can be baked into the single SPMD program.  Slots are grouped
    in quarters of 4; within a quarter all slots are padded to the same
    width (the quarter max, ~1% extra), keeping every access pattern
    regular.  The host un-shuffles output rows at the end.
  * b_sc is dropped (softmax shift invariance).  b_sa==0 takes a fused
    wide-ACT path; nonzero b_sa falls back to per-m-chunk ACTs with bias.
  * No max-subtraction in softmax: |scores| <= ||w_sc||_1 * max|tanh| so
    exp stays comfortably inside f32 range, and masked positions use the
    shift-invariant (s+1000)*mask trick whose exp underflows cleanly to 0.

Device pipeline, one slot (node-packed sample) at a time, fp16 matmuls
with f32 PSUM:

  mm1    hT[m, i] = sum_k W[k,m].T xT[k,i] over the slot's columns; tanh
         fused on ScalarE over 2-bank psum pairs -> th fp16.
  score  M=1 matvecs (1-col LDWEIGHTS is ~free): slot j's row accumulates
         at psum partition 32*(j%4) of its quarter's score tile via
         col-tiling (tile_position=(0,32*(j%4))) -- scores land spread
         across partitions with NO cross-partition move and NO DRAM
         bounce.  Engines only pay free-dim cost, so the softmax runs on
         the partition-strided [97, smax] view directly; garbage rows are
         masked to 0 (then exp(0-1000) == 0).
  smax   (s+1000)*mask on DVE, exp with accumulate on ScalarE, recip+mul.
  pool   attn -> PE transpose -> attnT columns; out[s] = sum_c
         attnT[:,c,32j].T @ xnat[s,c] with 4 samples per psum bank via
         col-tiling; fused tanh on the whole bank, DMA of the 4 rows.

Quarters are software-pipelined: each quarter's softmax+pooling tail is
emitted into the next quarter's matmul stream, so only the last quarter's
short (DMA-free) tail is exposed at the end.
"""

from contextlib import ExitStack

import numpy as np

import concourse.bass as bass
import concourse.bacc as bacc
import concourse.mybir as mybir
import concourse.tile as tile
from concourse.bass_utils import run_bass_kernel_spmd

F16 = mybir.dt.float16
F32 = mybir.dt.float32

B, N, H = 128, 512, 512
NCORES = 8
S = B // NCORES          # 16 samples (slots) per core
NQ = S // 4              # 4 quarters of 4 slots
KC = H // 128            # contraction chunks
MC = H // 128            # m chunks
SHIFT = 1000.0

Tanh = mybir.ActivationFunctionType.Tanh
Exp = mybir.ActivationFunctionType.Exp
Alu = mybir.AluOpType


def make_plan(node_mask):
    """Slot assignment + per-quarter widths (shared across cores)."""
    k = node_mask.astype(bool).sum(1)
    order = np.argsort(-k, kind="stable")
    qw = []
    for q in range(NQ):
        grp = k[order[q * 4 * NCORES:(q + 1) * 4 * NCORES]]
        qw.append(max(16, int(np.ceil(grp.max() / 16) * 16)))
    ncc = [(w + 127) // 128 for w in qw]
    return dict(order=order, qw=qw, ncc=ncc,
                cj_off=np.concatenate([[0], np.cumsum(np.repeat(ncc, 4))])
                .astype(int))


def build_program(plan, bsa_zero):
    qw = plan["qw"]
    ncc = plan["ncc"]
    cj_off = plan["cj_off"]
    cj_tot = int(cj_off[-1])

    nc = bacc.Bacc(trn_type="TRN2", target_bir_lowering=False,
                   num_devices=NCORES)

    # f32 const blob columns: bsa | ident97 | per-quarter masks (97 rows)
    CB_BSA, CB_ID = 0, MC
    CB_MSK = [CB_ID + 97]
    for q in range(NQ):
        CB_MSK.append(CB_MSK[-1] + qw[q])
    CB = CB_MSK[-1]

    xt_h = [nc.dram_tensor(f"xt{q}", [128, 4, KC, qw[q]], F16,
                           kind="ExternalInput") for q in range(NQ)]
    xn_h = [nc.dram_tensor(f"xn{q}", [128, 4 * ncc[q], H], F16,
                           kind="ExternalInput") for q in range(NQ)]
    wb_h = nc.dram_tensor("wblob", [128, MC * H + MC], F16,
                          kind="ExternalInput")
    cb_h = nc.dram_tensor("cblob", [128, CB], F32, kind="ExternalInput")
    out_h = nc.dram_tensor("out", [S, H], F32, kind="ExternalOutput")

    with tile.TileContext(nc) as tc, ExitStack() as ctx:
        const = ctx.enter_context(tc.tile_pool(name="const", bufs=1))
        xt_p = ctx.enter_context(tc.tile_pool(name="xt", bufs=1))
        xn_p = ctx.enter_context(tc.tile_pool(name="xn", bufs=1))
        th_p = ctx.enter_context(tc.tile_pool(name="th", bufs=6))
        sm_p = ctx.enter_context(tc.tile_pool(name="sm", bufs=1))
        row_p = ctx.enter_context(tc.tile_pool(name="row", bufs=2))
        ph_p = ctx.enter_context(tc.tile_pool(name="ph", bufs=2, space="PSUM"))
        pr_p = ctx.enter_context(tc.tile_pool(name="pr", bufs=2, space="PSUM"))
        pt_p = ctx.enter_context(tc.tile_pool(name="pt", bufs=2, space="PSUM"))

        # --- constants: few large DMAs (a dma_start costs ~0.6us of
        # issuing-engine time); weight blob split so m0 lands first
        wb = const.tile([128, MC * H + MC], F16, name="wb")
        cb = const.tile([128, CB], F32, name="cb")
        nc.scalar.dma_start(wb, wb_h.ap())
        nc.scalar.dma_start(cb, cb_h.ap())

        def Wf(k, m):
            return wb[:, m * H + k * 128:m * H + (k + 1) * 128]

        wsc = wb[:, MC * H:]
        bsa = cb[:, CB_BSA:CB_BSA + MC]
        idf = cb[:, CB_ID:CB_ID + 97]
        masks = [cb[0:97, CB_MSK[q]:CB_MSK[q + 1]] for q in range(NQ)]

        # HAM warmup: garbage matmuls on a memset tile (no DMA dependency)
        # keep PE activity continuous from ~7us into the real mm1 stream,
        # so the 2.4 GHz un-throttle fires before it starts
        wz = const.tile([128, 128], F16, name="wz")
        nc.vector.memset(wz, 0.0)
        nshift = const.tile([97, 1], F32, name="nshift")
        nc.vector.memset(nshift, -SHIFT)
        warm = pt_p.tile([128, H], F32, name="pp")
        for i in range(40):
            nc.tensor.matmul(warm[:, 0:128], lhsT=wz, rhs=wz,
                             start=(i == 0), stop=(i == 39))

        # --- x loads: per-quarter tensors, spread so each arrives well
        # before its quarter is consumed and xn never starves xt
        xt = [xt_p.tile([128, 4, KC, qw[q]], F16, name=f"xt{q}")
              for q in range(NQ)]
        xn = [xn_p.tile([128, 4 * ncc[q], H], F16, name=f"xn{q}")
              for q in range(NQ)]
        # each ~110GB/s ring carries transfers in consumption order; the
        # zero tail of each slot's last 128-node chunk is never sent (the
        # pooling matmul contracts only the first rem partitions there)
        def xn_load(q, queue):
            nf = ncc[q] - 1
            rem = qw[q] - 128 * nf
            if nf:
                queue.dma_start(
                    xn[q].rearrange("p (s c) h -> p s c h", c=ncc[q])
                    [:, :, 0:nf, :],
                    xn_h[q].ap().rearrange("p (s c) h -> p s c h", c=ncc[q])
                    [:, :, 0:nf, :])
            queue.dma_start(
                xn[q].rearrange("p (s c) h -> p s c h", c=ncc[q])
                [0:rem, :, nf, :],
                xn_h[q].ap().rearrange("p (s c) h -> p s c h", c=ncc[q])
                [0:rem, :, nf, :])

        # quarter 0 as per-slot singles on gpsimd (first bytes land ~10us);
        # later quarters split 2+2 across the fast rings in deadline order;
        # the slow sync ring carries only the two latest-needed xn blocks
        for jj in range(4):
            nc.gpsimd.dma_start(xt[0][:, jj], xt_h[0].ap()[:, jj])
        nc.sync.dma_start(xn[0], xn_h[0].ap())
        for q in (1, 2, 3):
            nc.scalar.dma_start(xt[q][:, 0:2], xt_h[q].ap()[:, 0:2])
            nc.gpsimd.dma_start(xt[q][:, 2:4], xt_h[q].ap()[:, 2:4])
        xn_load(1, nc.scalar)
        xn_load(3, nc.gpsimd)
        xn_load(2, nc.sync)

        def slot_mm(q, jj):
            """mm1 + fused tanh for one slot; returns the th tiles."""
            w = qw[q]
            ths = []
            for pair in range(2):
                # mi stride padded to 512 so each matmul's output stays
                # inside one 2KB PSUM bank -- a matmul that straddles a
                # bank boundary silently corrupts the spill region
                ph = ph_p.tile([128, 2, 512], F32, name="ph")
                for mi in range(2):
                    m = 2 * pair + mi
                    for k in range(KC):
                        nc.tensor.matmul(
                            ph[:, mi, 0:w],
                            lhsT=Wf(k, m),
                            rhs=xt[q][:, jj, k, :],
                            start=(k == 0), stop=(k == KC - 1),
                        )
                th = th_p.tile([128, 2, 288], F16, name="th")
                if bsa_zero:
                    nc.scalar.activation(th[:, :, 0:w], ph[:, :, 0:w], Tanh)
                else:
                    for mi in range(2):
                        m = 2 * pair + mi
                        nc.scalar.activation(th[:, mi, 0:w], ph[:, mi, 0:w],
                                             Tanh, bias=bsa[:, m:m + 1])
                ths.append(th)
            return ths

        def slot_scores(pair):
            """col-tiled score rows for two slots, deferred behind their
            mm1 (so tanh is long done) and interleaved m-major: the two
            slots sit on different col strips, so their M=1 matmuls run
            concurrently in the array."""
            for m in range(MC):
                for q, jj, ths, pscore in pair:
                    nc.tensor.matmul(
                        pscore[32 * jj:32 * jj + 1, 0:qw[q]],
                        lhsT=wsc[:, m:m + 1],
                        rhs=ths[m // 2][:, m % 2, 0:qw[q]],
                        start=(m == 0), stop=(m == MC - 1),
                        tile_position=(0, 32 * jj),
                        skip_group_check=True,
                    )

        def tail_soft(q, pscore):
            """softmax on the partition-strided score tile (no PE work).
            Rows between the 4 live ones hold garbage; the mask zeroes
            them ((g+SHIFT)*0 = 0) and exp(0-SHIFT) underflows to 0."""
            w = qw[q]
            sco = sm_p.tile([97, w], F32, name=f"sco{q}")
            nc.vector.tensor_copy(sco, pscore[0:97, 0:w])
            m1 = sm_p.tile([97, w], F32, name=f"m1_{q}")
            nc.vector.scalar_tensor_tensor(m1, sco, SHIFT, masks[q],
                                           op0=Alu.add, op1=Alu.mult)
            ex = sm_p.tile([97, w], F32, name=f"ex{q}")
            esum = sm_p.tile([97, 1], F32, name=f"esum{q}")
            nc.scalar.activation(ex, m1, Exp, bias=nshift, accum_out=esum)
            rinv = sm_p.tile([97, 1], F32, name=f"rinv{q}")
            nc.vector.reciprocal(rinv, esum)
            attn = sm_p.tile([97, w], F32, name=f"attn{q}")
            nc.vector.tensor_scalar_mul(attn, ex, rinv)
            return attn

        def tail_pool(q, attn):
            """attn transpose + col-tiled pooling + output (PE phase)."""
            w = qw[q]
            paT = pt_p.tile([128, H], F32, name="pp")
            for c in range(ncc[q]):
                wcol = min(128, w - c * 128)
                nc.tensor.transpose(paT[0:wcol, c * 97:c * 97 + 97],
                                    attn[:, c * 128:c * 128 + wcol],
                                    idf[0:97, :])
            attnT = sm_p.tile([128, ncc[q], 97], F16, name=f"attnT{q}")
            nc.vector.tensor_copy(attnT, paT[:, 0:ncc[q] * 97])

            pp = pt_p.tile([128, H], F32, name="pp")
            for c in range(ncc[q]):
                # last chunk contracts only its real rem partitions (the
                # zero tail is neither loaded nor touched)
                pn = min(128, w - 128 * c)
                for jj in range(4):
                    nc.tensor.matmul(
                        pp[32 * jj:32 * jj + 1, :],
                        lhsT=attnT[0:pn, c, 32 * jj:32 * jj + 1],
                        rhs=xn[q][0:pn, jj * ncc[q] + c, :],
                        start=(c == 0), stop=(c == ncc[q] - 1),
                        tile_position=(0, 32 * jj),
                    )
            orow = row_p.tile([128, H], F32, name="orow")
            nc.scalar.activation(orow, pp, Tanh)
            nc.sync.dma_start(
                out_h.ap().rearrange("(q four) h -> four q h", four=4)
                [:, q, :],
                orow[0:97:32, :])

        # pipeline: slot scores trail their mm1 by one slot, a quarter's
        # softmax follows its last scores, and its PE pooling is deferred
        # into the middle of the next quarter's matmul stream
        # pooling runs two quarters behind its softmax so the xn transfers
        # (on the slower rings) have relaxed deadlines
        attns = {}
        pscores = {}
        backlog = []
        ready = []

        def flush(n):
            pair = backlog[:n]
            del backlog[:n]
            slot_scores(pair)
            for pq, pjj, _, _ in pair:
                if pjj == 3:
                    attns[pq] = tail_soft(pq, pscores.pop(pq))
                    ready.append(pq)

        for q in range(NQ):
            pscores[q] = pr_p.tile([128, 512], F32, name="pscore")
            for jj in range(4):
                ths = slot_mm(q, jj)
                backlog.append((q, jj, ths, pscores[q]))
                if len(backlog) == 3:
                    flush(2)
                if jj == 2 and ready:
                    pq2 = ready.pop(0)
                    tail_pool(pq2, attns.pop(pq2))
        flush(len(backlog))
        for pq in ready:
            tail_pool(pq, attns.pop(pq))

    nc.finalize()
    return nc


_CACHE = {}


def _get_nc(plan, bsa_zero):
    key = (tuple(plan["qw"]), bsa_zero)
    if key not in _CACHE:
        _CACHE[key] = build_program(plan, bsa_zero)
    return _CACHE[key]


def make_in_maps(plan, code_feat, node_mask, W_sa, b_sa, w_sc):
    x16 = np.asarray(code_feat, dtype=np.float16)
    kmask = np.asarray(node_mask).astype(bool)
    order = plan["order"]
    qw, ncc = plan["qw"], plan["ncc"]

    w16 = np.asarray(W_sa, dtype=np.float16)
    wblob = np.empty((128, MC * H + MC), dtype=np.float16)
    wblob[:, 0:MC * H] = (w16.reshape(KC, 128, MC, 128)
                          .transpose(1, 2, 0, 3).reshape(128, MC * H))
    wblob[:, MC * H:] = np.asarray(w_sc, dtype=np.float16).reshape(MC, 128).T

    CB = MC + 97 + sum(qw)
    cblob0 = np.zeros((128, CB), dtype=np.float32)
    cblob0[:, 0:MC] = np.asarray(b_sa, dtype=np.float32).reshape(MC, 128).T
    cblob0[0:97, MC:MC + 97] = np.eye(97, dtype=np.float32)
    qcol = np.concatenate([[MC + 97], MC + 97 + np.cumsum(qw)]).astype(int)

    in_maps = []
    for i in range(NCORES):
        im = {"wblob": wblob}
        cblob = cblob0.copy()
        for q in range(NQ):
            w, nc_q = qw[q], ncc[q]
            xtq = np.zeros((4, w, H), dtype=np.float16)
            xnq = np.zeros((4, nc_q * 128, H), dtype=np.float16)
            for jj in range(4):
                j = q * 4 + jj
                s = order[j * NCORES + i]
                idx = np.nonzero(kmask[s])[0]
                xtq[jj, 0:len(idx)] = x16[s, idx]
                xnq[jj, 0:len(idx)] = x16[s, idx]
                cblob[32 * jj, qcol[q]:qcol[q] + len(idx)] = 1.0
            # xt: [p(h%128), slot, k, i]
            im[f"xt{q}"] = np.ascontiguousarray(
                xtq.transpose(0, 2, 1).reshape(4, KC, 128, w)
                .transpose(2, 0, 1, 3))
            # xn: [p(n%128), slot*ncc + c, h]
            im[f"xn{q}"] = np.ascontiguousarray(
                xnq.reshape(4 * nc_q, 128, H).transpose(1, 0, 2))
            im["cblob"] = cblob
        in_maps.append(im)
    return in_maps


def kernel(code_feat, node_mask, W_sa, b_sa, w_sc, b_sc=None, **_ignored):
    code_feat = np.asarray(code_feat)
    node_mask = np.asarray(node_mask)
    plan = make_plan(node_mask)
    bsa_zero = not np.any(np.asarray(b_sa))
    nc = _get_nc(plan, bsa_zero)
    in_maps = make_in_maps(plan, code_feat, node_mask, W_sa, b_sa, w_sc)
    res = run_bass_kernel_spmd(nc, in_maps, list(range(NCORES)))
    out = np.empty((B, H), dtype=np.float32)
    order = plan["order"]
    for i in range(NCORES):
        for j in range(S):
            out[order[j * NCORES + i]] = res.results[i]["out"][j]
    return out
